# revision 4
# baseline (speedup 1.0000x reference)
"""Birman-Schwinger core: K[b] = diag(sqrt|V_b|) @ R_0 @ diag(sqrt|V_b|).

Key identity: with g[b,u] = sqrt(|V[b,u]| + eps) / (1 + u) and d = u - v,

    K[b,u,v] = g[b,u] * g[b,v] * H(d)
    H(d) = 0.5j * exp(2j*d) * sign(d)
         = -0.5*sign(d)*sin(2d)  +  0.5j*sign(d)*cos(2d)

H is a fixed Toeplitz table, so each 128-row block of the output is an
elementwise product of a sliding window of the (host-precomputed) H table,
a per-partition scalar g_u, and a broadcast row g_v — one fused
scalar_tensor_tensor op on the Vector engine per tile, then DMA out.

Sharding: 8 cores; core c handles batch b = c // 2, row half h = c % 2
(rows [2048*h, 2048*h + 2048) of the (4096, 4096) complex output).
Output is written as interleaved re/im f32 pairs so the per-core
(2048, 8192) f32 result is exactly the complex64 memory layout.
"""

import numpy as np

B = 4
N = 4096
NCORES = 8
HALF = N // 2            # rows per core
P = 128                  # SBUF partitions
NBLK = HALF // P         # 16 row blocks per core
EPS = 1e-10
MW = 4096 + 15 * P       # master table width in complex columns (6016)
SHIFT = 15 * P           # block k slice starts at complex col SHIFT - P*k
CHUNK = 4096             # f32 columns per compute/DMA tile (2048 complex)
NCHUNK = (2 * N) // CHUNK

_PROGRAM_CACHE = {}


def _build_program():
    import concourse.bacc as bacc
    import concourse.mybir as mybir
    from concourse.tile import TileContext

    nc = bacc.Bacc("TRN2", target_bir_lowering=False, debug=False)
    m = nc.dram_tensor("t_m", [P, 2 * MW], mybir.dt.float32, kind="ExternalInput").ap()
    gv = nc.dram_tensor("t_gv", [1, 2 * N], mybir.dt.float32, kind="ExternalInput").ap()
    gu = nc.dram_tensor("t_gu", [P, NBLK], mybir.dt.float32, kind="ExternalInput").ap()
    out = nc.dram_tensor(
        "t_out", [HALF, 2 * N], mybir.dt.float32, kind="ExternalOutput"
    ).ap()
    mult = mybir.AluOpType.mult

    with TileContext(nc) as tc:
        with (
            tc.tile_pool(name="const", bufs=1) as cpool,
            tc.tile_pool(name="work", bufs=4) as wpool,
            tc.tile_pool(name="tmp", bufs=2) as tpool,
        ):
            m_sb = cpool.tile([P, 2 * MW], mybir.dt.float32)
            gvb_sb = cpool.tile([P, 2 * N], mybir.dt.float32)
            gu_sb = cpool.tile([P, NBLK], mybir.dt.float32)
            nc.sync.dma_start(out=gu_sb[:, :], in_=gu[:, :])
            # g_v broadcast row: 32KB DMA into partition 0 + on-chip
            # partition replication instead of a 4MB HBM load.
            nc.sync.dma_start(out=gvb_sb[0:1, :], in_=gv[:, :])
            nc.gpsimd.partition_broadcast(gvb_sb[:, :], gvb_sb[0:1, :])
            # Load the master table in two column chunks; block k = NBLK-1
            # reads cols [0, 2N) only, so descending-k compute can start
            # before the high columns land.
            nc.sync.dma_start(out=m_sb[:, : 2 * N], in_=m[:, : 2 * N])
            nc.sync.dma_start(out=m_sb[:, 2 * N :], in_=m[:, 2 * N :])

            ci = 0
            for k in reversed(range(NBLK)):
                s2 = 2 * (SHIFT - P * k)  # f32 col offset of block k's window
                for j in range(NCHUNK):
                    m_slice = m_sb[:, s2 + j * CHUNK : s2 + (j + 1) * CHUNK]
                    gvb_slice = gvb_sb[:, j * CHUNK : (j + 1) * CHUNK]
                    gu_scal = gu_sb[:, k : k + 1]
                    t = wpool.tile([P, CHUNK], mybir.dt.float32)
                    if ci % 3 == 2:
                        # Offload ~1/3 of the elementwise work: GpSimd does
                        # the tensor*tensor, ScalarE applies the per-partition
                        # scale, keeping the Vector engine ahead of the
                        # HBM store stream.
                        tt = tpool.tile([P, CHUNK], mybir.dt.float32)
                        nc.gpsimd.tensor_tensor(
                            out=tt[:, :], in0=m_slice, in1=gvb_slice, op=mult
                        )
                        nc.scalar.activation(
                            out=t[:, :],
                            in_=tt[:, :],
                            func=mybir.ActivationFunctionType.Copy,
                            scale=gu_scal,
                        )
                    else:
                        nc.vector.scalar_tensor_tensor(
                            out=t[:, :],
                            in0=m_slice,
                            scalar=gu_scal,
                            in1=gvb_slice,
                            op0=mult,
                            op1=mult,
                        )
                    nc.sync.dma_start(
                        out=out[k * P : (k + 1) * P, j * CHUNK : (j + 1) * CHUNK],
                        in_=t[:, :],
                    )
                    ci += 1
    nc.compile()
    return nc


def _get_program():
    if "nc" not in _PROGRAM_CACHE:
        _PROGRAM_CACHE["nc"] = _build_program()
    return _PROGRAM_CACHE["nc"]


def _host_tables(V):
    """Per-core input arrays (all f32)."""
    pos = np.arange(N, dtype=np.float64)
    g = (np.sqrt(np.abs(V).astype(np.float64) + EPS) / (1.0 + pos)).astype(
        np.float32
    )  # (B, N)

    masters = {}
    for h in range(2):
        u0 = HALF * h
        p = np.arange(P, dtype=np.int64)[:, None]
        j = np.arange(MW, dtype=np.int64)[None, :]
        d = p - j + u0 + SHIFT
        s = np.sign(d).astype(np.float64)
        hre = -0.5 * s * np.sin(2.0 * d)
        him = 0.5 * s * np.cos(2.0 * d)
        inter = np.empty((P, 2 * MW), dtype=np.float32)
        inter[:, 0::2] = hre
        inter[:, 1::2] = him
        masters[h] = inter

    in_maps = []
    for c in range(NCORES):
        b, h = divmod(c, 2)
        u0 = HALF * h
        grow = np.empty((1, 2 * N), dtype=np.float32)
        grow[0, 0::2] = g[b]
        grow[0, 1::2] = g[b]
        gu = np.ascontiguousarray(g[b, u0 : u0 + HALF].reshape(NBLK, P).T)
        in_maps.append({"t_m": masters[h], "t_gv": grow, "t_gu": gu})
    return in_maps


def _run(in_maps, trace=False, **kwargs):
    from concourse import bass_utils

    nc = _get_program()
    return bass_utils.run_bass_kernel_spmd(
        nc, in_maps, core_ids=list(range(NCORES)), trace=trace, **kwargs
    )


def kernel(V):
    V = np.asarray(V)
    assert V.shape == (B, N), V.shape
    in_maps = _host_tables(V)
    res = _run(in_maps, trace=False)
    out = np.empty((B, N, N), dtype=np.complex64)
    for c in range(NCORES):
        b, h = divmod(c, 2)
        plane = np.ascontiguousarray(res.results[c]["t_out"])
        out[b, HALF * h : HALF * (h + 1), :] = plane.view(np.complex64)
    return out


# revision 6
# speedup vs baseline: 1.0647x; 1.0647x over previous
"""Birman-Schwinger core: K[b] = diag(sqrt|V_b|) @ R_0 @ diag(sqrt|V_b|).

Key identity: with g[b,u] = sqrt(|V[b,u]| + eps) / (1 + u) and d = u - v,

    K[b,u,v] = g[b,u] * g[b,v] * H(d)
    H(d) = 0.5j * exp(2j*d) * sign(d)
         = -0.5*sign(d)*sin(2d)  +  0.5j*sign(d)*cos(2d)

H is a fixed Toeplitz table, so each 128-row block of the output is an
elementwise product of a sliding window of the (host-precomputed) H table,
a per-partition scalar g_u, and a broadcast row g_v — one fused
scalar_tensor_tensor op on the Vector engine per tile, then DMA out.

Sharding: 8 cores; core c handles batch b = c // 2, row half h = c % 2
(rows [2048*h, 2048*h + 2048) of the (4096, 4096) complex output).
Output is written as interleaved re/im f32 pairs so the per-core
(2048, 8192) f32 result is exactly the complex64 memory layout.
"""

import numpy as np

B = 4
N = 4096
NCORES = 8
HALF = N // 2            # rows per core
P = 128                  # SBUF partitions
NBLK = HALF // P         # 16 row blocks per core
EPS = 1e-10
MW = 4096 + 15 * P       # master table width in complex columns (6016)
SHIFT = 15 * P           # block k slice starts at complex col SHIFT - P*k
CHUNK = 8192             # f32 columns per compute/DMA tile (4096 complex)
NCHUNK = (2 * N) // CHUNK

_PROGRAM_CACHE = {}


def _build_program():
    import concourse.bacc as bacc
    import concourse.mybir as mybir
    from concourse.tile import TileContext

    nc = bacc.Bacc("TRN2", target_bir_lowering=False, debug=False)
    m = nc.dram_tensor("t_m", [P, 2 * MW], mybir.dt.float32, kind="ExternalInput").ap()
    gv = nc.dram_tensor("t_gv", [1, 2 * N], mybir.dt.float32, kind="ExternalInput").ap()
    gu = nc.dram_tensor("t_gu", [P, NBLK], mybir.dt.float32, kind="ExternalInput").ap()
    out = nc.dram_tensor(
        "t_out", [HALF, 2 * N], mybir.dt.float32, kind="ExternalOutput"
    ).ap()
    mult = mybir.AluOpType.mult

    with TileContext(nc) as tc:
        with (
            tc.tile_pool(name="const", bufs=1) as cpool,
            tc.tile_pool(name="work", bufs=3) as wpool,
        ):
            m_sb = cpool.tile([P, 2 * MW], mybir.dt.float32)
            gvb_sb = cpool.tile([P, 2 * N], mybir.dt.float32)
            gu_sb = cpool.tile([P, NBLK], mybir.dt.float32)
            nc.sync.dma_start(out=gu_sb[:, :], in_=gu[:, :])
            # g_v broadcast row: 32KB DMA into partition 0 + on-chip
            # partition replication instead of a 4MB HBM load.
            nc.sync.dma_start(out=gvb_sb[0:1, :], in_=gv[:, :])
            nc.gpsimd.partition_broadcast(gvb_sb[:, :], gvb_sb[0:1, :])
            # Load the master table in column chunks; block k = NBLK-1 reads
            # cols [0, 2N) only, so descending-k compute starts before the
            # high columns land.
            for q in range(4):
                w = (2 * MW) // 4
                nc.sync.dma_start(
                    out=m_sb[:, q * w : (q + 1) * w], in_=m[:, q * w : (q + 1) * w]
                )

            ci = 0
            for k in reversed(range(NBLK)):
                s2 = 2 * (SHIFT - P * k)  # f32 col offset of block k's window
                for j in range(NCHUNK):
                    t = wpool.tile([P, CHUNK], mybir.dt.float32)
                    nc.vector.scalar_tensor_tensor(
                        out=t[:, :],
                        in0=m_sb[:, s2 + j * CHUNK : s2 + (j + 1) * CHUNK],
                        scalar=gu_sb[:, k : k + 1],
                        in1=gvb_sb[:, j * CHUNK : (j + 1) * CHUNK],
                        op0=mult,
                        op1=mult,
                    )
                    # Alternate output DMAs across the two HWDGE rings
                    # (SP and ACT) for more in-flight descriptors.
                    dma_eng = nc.sync if ci % 2 == 0 else nc.scalar
                    dma_eng.dma_start(
                        out=out[k * P : (k + 1) * P, j * CHUNK : (j + 1) * CHUNK],
                        in_=t[:, :],
                    )
                    ci += 1
    nc.compile()
    return nc


def _get_program():
    if "nc" not in _PROGRAM_CACHE:
        _PROGRAM_CACHE["nc"] = _build_program()
    return _PROGRAM_CACHE["nc"]


def _host_tables(V):
    """Per-core input arrays (all f32)."""
    pos = np.arange(N, dtype=np.float64)
    g = (np.sqrt(np.abs(V).astype(np.float64) + EPS) / (1.0 + pos)).astype(
        np.float32
    )  # (B, N)

    masters = {}
    for h in range(2):
        u0 = HALF * h
        p = np.arange(P, dtype=np.int64)[:, None]
        j = np.arange(MW, dtype=np.int64)[None, :]
        d = p - j + u0 + SHIFT
        s = np.sign(d).astype(np.float64)
        hre = -0.5 * s * np.sin(2.0 * d)
        him = 0.5 * s * np.cos(2.0 * d)
        inter = np.empty((P, 2 * MW), dtype=np.float32)
        inter[:, 0::2] = hre
        inter[:, 1::2] = him
        masters[h] = inter

    in_maps = []
    for c in range(NCORES):
        b, h = divmod(c, 2)
        u0 = HALF * h
        grow = np.empty((1, 2 * N), dtype=np.float32)
        grow[0, 0::2] = g[b]
        grow[0, 1::2] = g[b]
        gu = np.ascontiguousarray(g[b, u0 : u0 + HALF].reshape(NBLK, P).T)
        in_maps.append({"t_m": masters[h], "t_gv": grow, "t_gu": gu})
    return in_maps


def _run(in_maps, trace=False, **kwargs):
    from concourse import bass_utils

    nc = _get_program()
    return bass_utils.run_bass_kernel_spmd(
        nc, in_maps, core_ids=list(range(NCORES)), trace=trace, **kwargs
    )


def kernel(V):
    V = np.asarray(V)
    assert V.shape == (B, N), V.shape
    in_maps = _host_tables(V)
    res = _run(in_maps, trace=False)
    out = np.empty((B, N, N), dtype=np.complex64)
    for c in range(NCORES):
        b, h = divmod(c, 2)
        plane = np.ascontiguousarray(res.results[c]["t_out"])
        out[b, HALF * h : HALF * (h + 1), :] = plane.view(np.complex64)
    return out


# revision 8
# speedup vs baseline: 1.1042x; 1.0372x over previous
"""Birman-Schwinger core: K[b] = diag(sqrt|V_b|) @ R_0 @ diag(sqrt|V_b|).

Key identity: with g[b,u] = sqrt(|V[b,u]| + eps) / (1 + u) and d = u - v,

    K[b,u,v] = g[b,u] * g[b,v] * H(d)
    H(d) = 0.5j * exp(2j*d) * sign(d)
         = -0.5*sign(d)*sin(2d)  +  0.5j*sign(d)*cos(2d)

H is a fixed Toeplitz table, so each 128-row block of the output is an
elementwise product of a sliding window of the (host-precomputed) H table,
a per-partition scalar g_u, and a broadcast row g_v — one fused
scalar_tensor_tensor op on the Vector engine per tile, then DMA out.

Sharding: 8 cores; core c handles batch b = c // 2, row half h = c % 2
(rows [2048*h, 2048*h + 2048) of the (4096, 4096) complex output).
Output is written as interleaved re/im f32 pairs so the per-core
(2048, 8192) f32 result is exactly the complex64 memory layout.
"""

import numpy as np

B = 4
N = 4096
NCORES = 8
HALF = N // 2            # rows per core
P = 128                  # SBUF partitions
NBLK = HALF // P         # 16 row blocks per core
EPS = 1e-10
MW = 4096 + 15 * P       # master table width in complex columns (6016)
SHIFT = 15 * P           # block k slice starts at complex col SHIFT - P*k
CHUNK = 4096             # f32 columns per compute/DMA tile (2048 complex)
NCHUNK = (2 * N) // CHUNK

_PROGRAM_CACHE = {}


def _build_program():
    import concourse.bacc as bacc
    import concourse.mybir as mybir
    from concourse.tile import TileContext

    nc = bacc.Bacc("TRN2", target_bir_lowering=False, debug=False)
    m = nc.dram_tensor("t_m", [P, 2 * MW], mybir.dt.float32, kind="ExternalInput").ap()
    gv = nc.dram_tensor("t_gv", [1, 2 * N], mybir.dt.float32, kind="ExternalInput").ap()
    gu = nc.dram_tensor("t_gu", [P, NBLK], mybir.dt.float32, kind="ExternalInput").ap()
    out = nc.dram_tensor(
        "t_out", [HALF, 2 * N], mybir.dt.float32, kind="ExternalOutput"
    ).ap()
    mult = mybir.AluOpType.mult

    with TileContext(nc) as tc:
        with (
            tc.tile_pool(name="const", bufs=1) as cpool,
            tc.tile_pool(name="work", bufs=4) as wpool,
        ):
            m_sb = cpool.tile([P, 2 * MW], mybir.dt.float32)
            gvb_sb = cpool.tile([P, 2 * N], mybir.dt.float32)
            gu_sb = cpool.tile([P, NBLK], mybir.dt.float32)
            # g_v broadcast row: 32KB DMA into partition 0 + on-chip
            # partition replication (in halves, so the first compute chunk
            # unblocks early) instead of a 4MB HBM load.
            nc.sync.dma_start(out=gvb_sb[0:1, :], in_=gv[:, :])
            nc.sync.dma_start(out=gu_sb[:, :], in_=gu[:, :])
            nc.gpsimd.partition_broadcast(
                gvb_sb[:, :CHUNK], gvb_sb[0:1, :CHUNK]
            )
            nc.gpsimd.partition_broadcast(
                gvb_sb[:, CHUNK:], gvb_sb[0:1, CHUNK:]
            )
            # Load the master table in consumption order (descending-k blocks
            # read columns low to high); the first compute chunk only needs
            # cols [0, CHUNK).
            for q0 in range(0, 2 * MW, CHUNK):
                q1 = min(q0 + CHUNK, 2 * MW)
                nc.sync.dma_start(out=m_sb[:, q0:q1], in_=m[:, q0:q1])

            ci = 0
            for k in reversed(range(NBLK)):
                s2 = 2 * (SHIFT - P * k)  # f32 col offset of block k's window
                for j in range(NCHUNK):
                    t = wpool.tile([P, CHUNK], mybir.dt.float32)
                    nc.vector.scalar_tensor_tensor(
                        out=t[:, :],
                        in0=m_sb[:, s2 + j * CHUNK : s2 + (j + 1) * CHUNK],
                        scalar=gu_sb[:, k : k + 1],
                        in1=gvb_sb[:, j * CHUNK : (j + 1) * CHUNK],
                        op0=mult,
                        op1=mult,
                    )
                    # Alternate output DMAs across the two HWDGE rings
                    # (SP and ACT) for more in-flight descriptors.
                    dma_eng = nc.sync if ci % 2 == 0 else nc.scalar
                    dma_eng.dma_start(
                        out=out[k * P : (k + 1) * P, j * CHUNK : (j + 1) * CHUNK],
                        in_=t[:, :],
                    )
                    ci += 1
    nc.compile()
    return nc


def _get_program():
    if "nc" not in _PROGRAM_CACHE:
        _PROGRAM_CACHE["nc"] = _build_program()
    return _PROGRAM_CACHE["nc"]


def _host_tables(V):
    """Per-core input arrays (all f32)."""
    pos = np.arange(N, dtype=np.float64)
    g = (np.sqrt(np.abs(V).astype(np.float64) + EPS) / (1.0 + pos)).astype(
        np.float32
    )  # (B, N)

    masters = {}
    for h in range(2):
        u0 = HALF * h
        p = np.arange(P, dtype=np.int64)[:, None]
        j = np.arange(MW, dtype=np.int64)[None, :]
        d = p - j + u0 + SHIFT
        s = np.sign(d).astype(np.float64)
        hre = -0.5 * s * np.sin(2.0 * d)
        him = 0.5 * s * np.cos(2.0 * d)
        inter = np.empty((P, 2 * MW), dtype=np.float32)
        inter[:, 0::2] = hre
        inter[:, 1::2] = him
        masters[h] = inter

    in_maps = []
    for c in range(NCORES):
        b, h = divmod(c, 2)
        u0 = HALF * h
        grow = np.empty((1, 2 * N), dtype=np.float32)
        grow[0, 0::2] = g[b]
        grow[0, 1::2] = g[b]
        gu = np.ascontiguousarray(g[b, u0 : u0 + HALF].reshape(NBLK, P).T)
        in_maps.append({"t_m": masters[h], "t_gv": grow, "t_gu": gu})
    return in_maps


def _run(in_maps, trace=False, **kwargs):
    from concourse import bass_utils

    nc = _get_program()
    return bass_utils.run_bass_kernel_spmd(
        nc, in_maps, core_ids=list(range(NCORES)), trace=trace, **kwargs
    )


def kernel(V):
    V = np.asarray(V)
    assert V.shape == (B, N), V.shape
    in_maps = _host_tables(V)
    res = _run(in_maps, trace=False)
    out = np.empty((B, N, N), dtype=np.complex64)
    for c in range(NCORES):
        b, h = divmod(c, 2)
        plane = np.ascontiguousarray(res.results[c]["t_out"])
        out[b, HALF * h : HALF * (h + 1), :] = plane.view(np.complex64)
    return out


# revision 9
# speedup vs baseline: 1.2982x; 1.1757x over previous
"""Birman-Schwinger core: K[b] = diag(sqrt|V_b|) @ R_0 @ diag(sqrt|V_b|).

Key identity: with g[b,u] = sqrt(|V[b,u]| + eps) / (1 + u) and d = u - v,

    K[b,u,v] = g[b,u] * g[b,v] * H(d)
    H(d) = 0.5j * exp(2j*d) * sign(d)
         = -0.5*sign(d)*sin(2d)  +  0.5j*sign(d)*cos(2d)

H is a fixed Toeplitz table, so each 128-row block of the output is an
elementwise product of a sliding window of the (host-precomputed) H table,
a per-partition scalar g_u, and a broadcast row g_v — one fused
scalar_tensor_tensor op on the Vector engine per tile, then DMA out.

Sharding: 8 cores; core c handles batch b = c // 2, row half h = c % 2
(rows [2048*h, 2048*h + 2048) of the (4096, 4096) complex output).
Output is written as interleaved re/im f32 pairs so the per-core
(2048, 8192) f32 result is exactly the complex64 memory layout.
"""

import numpy as np

B = 4
N = 4096
NCORES = 8
HALF = N // 2            # rows per core
P = 128                  # SBUF partitions
NBLK = HALF // P         # 16 row blocks per core
EPS = 1e-10
MW = 4096 + 15 * P       # master table width in complex columns (6016)
SHIFT = 15 * P           # block k slice starts at complex col SHIFT - P*k
CHUNK = 4096             # f32 columns per compute/DMA tile (2048 complex)
NCHUNK = (2 * N) // CHUNK

_PROGRAM_CACHE = {}


def _build_program():
    import concourse.bacc as bacc
    import concourse.mybir as mybir
    from concourse.tile import TileContext

    nc = bacc.Bacc("TRN2", target_bir_lowering=False, debug=False)
    m = nc.dram_tensor("t_m", [P, 2 * MW], mybir.dt.float32, kind="ExternalInput").ap()
    gv = nc.dram_tensor("t_gv", [1, 2 * N], mybir.dt.float32, kind="ExternalInput").ap()
    gu = nc.dram_tensor("t_gu", [P, NBLK], mybir.dt.float32, kind="ExternalInput").ap()
    out = nc.dram_tensor(
        "t_out", [HALF, 2 * N], mybir.dt.float32, kind="ExternalOutput"
    ).ap()
    mult = mybir.AluOpType.mult

    with TileContext(nc) as tc:
        with (
            tc.tile_pool(name="const", bufs=1) as cpool,
            tc.tile_pool(name="work", bufs=6) as wpool,
        ):
            m_sb = cpool.tile([P, 2 * MW], mybir.dt.float32)
            gvb_sb = cpool.tile([P, 2 * N], mybir.dt.float32)
            gu_sb = cpool.tile([P, NBLK], mybir.dt.float32)
            # g_v broadcast row: 32KB DMA into partition 0 + on-chip
            # partition replication (in halves, so the first compute chunk
            # unblocks early) instead of a 4MB HBM load.
            nc.sync.dma_start(out=gvb_sb[0:1, :], in_=gv[:, :])
            nc.sync.dma_start(out=gu_sb[:, :], in_=gu[:, :])
            nc.gpsimd.partition_broadcast(
                gvb_sb[:, :CHUNK], gvb_sb[0:1, :CHUNK]
            )
            nc.gpsimd.partition_broadcast(
                gvb_sb[:, CHUNK:], gvb_sb[0:1, CHUNK:]
            )
            # Load the master table in consumption order (descending-k blocks
            # read columns low to high); the first compute chunk only needs
            # cols [0, CHUNK).
            for q0 in range(0, 2 * MW, CHUNK):
                q1 = min(q0 + CHUNK, 2 * MW)
                nc.sync.dma_start(out=m_sb[:, q0:q1], in_=m[:, q0:q1])

            ci = 0
            for k in reversed(range(NBLK)):
                s2 = 2 * (SHIFT - P * k)  # f32 col offset of block k's window
                for j in range(NCHUNK):
                    t = wpool.tile([P, CHUNK], mybir.dt.float32)
                    nc.vector.scalar_tensor_tensor(
                        out=t[:, :],
                        in0=m_sb[:, s2 + j * CHUNK : s2 + (j + 1) * CHUNK],
                        scalar=gu_sb[:, k : k + 1],
                        in1=gvb_sb[:, j * CHUNK : (j + 1) * CHUNK],
                        op0=mult,
                        op1=mult,
                    )
                    # Alternate output DMAs across the two HWDGE rings
                    # (SP and ACT) for more in-flight descriptors.
                    dma_eng = nc.sync if ci % 2 == 0 else nc.scalar
                    dma_eng.dma_start(
                        out=out[k * P : (k + 1) * P, j * CHUNK : (j + 1) * CHUNK],
                        in_=t[:, :],
                    )
                    ci += 1
    nc.compile()
    return nc


def _get_program():
    if "nc" not in _PROGRAM_CACHE:
        _PROGRAM_CACHE["nc"] = _build_program()
    return _PROGRAM_CACHE["nc"]


def _host_tables(V):
    """Per-core input arrays (all f32)."""
    pos = np.arange(N, dtype=np.float64)
    g = (np.sqrt(np.abs(V).astype(np.float64) + EPS) / (1.0 + pos)).astype(
        np.float32
    )  # (B, N)

    masters = {}
    for h in range(2):
        u0 = HALF * h
        p = np.arange(P, dtype=np.int64)[:, None]
        j = np.arange(MW, dtype=np.int64)[None, :]
        d = p - j + u0 + SHIFT
        s = np.sign(d).astype(np.float64)
        hre = -0.5 * s * np.sin(2.0 * d)
        him = 0.5 * s * np.cos(2.0 * d)
        inter = np.empty((P, 2 * MW), dtype=np.float32)
        inter[:, 0::2] = hre
        inter[:, 1::2] = him
        masters[h] = inter

    in_maps = []
    for c in range(NCORES):
        b, h = divmod(c, 2)
        u0 = HALF * h
        grow = np.empty((1, 2 * N), dtype=np.float32)
        grow[0, 0::2] = g[b]
        grow[0, 1::2] = g[b]
        gu = np.ascontiguousarray(g[b, u0 : u0 + HALF].reshape(NBLK, P).T)
        in_maps.append({"t_m": masters[h], "t_gv": grow, "t_gu": gu})
    return in_maps


def _run(in_maps, trace=False, **kwargs):
    from concourse import bass_utils

    nc = _get_program()
    return bass_utils.run_bass_kernel_spmd(
        nc, in_maps, core_ids=list(range(NCORES)), trace=trace, **kwargs
    )


def kernel(V):
    V = np.asarray(V)
    assert V.shape == (B, N), V.shape
    in_maps = _host_tables(V)
    res = _run(in_maps, trace=False)
    out = np.empty((B, N, N), dtype=np.complex64)
    for c in range(NCORES):
        b, h = divmod(c, 2)
        plane = np.ascontiguousarray(res.results[c]["t_out"])
        out[b, HALF * h : HALF * (h + 1), :] = plane.view(np.complex64)
    return out
